# revision 20
# baseline (speedup 1.0000x reference)
"""Trainium2 Bass kernel for the LNN Euler-Lagrange residual.

Math: for a ReLU MLP Lagrangian L(q, qdot) the JAX second-derivative term
d/dt(dL/dqdot) is identically zero (piecewise-linear network), so the
reference output reduces to -dL/dq:

    z1 = x @ W1 + b1          s1 = z1 > 0      a1 = relu(z1)
    z2 = a1 @ W2 + b2         s2 = z2 > 0
    pre1 = s2 @ W2T_eff       (W2T_eff[j,i] = w3[j] * W2[i, j])
    out  = (pre1 * s1) @ (-W1[:32,:].T)

Layout: feature-major (features on partitions, batch streams as matmul
free dim). Host pre-transposes the input shard to [64, B_core]. Two
batch groups are packed on the 128 partitions via host-built 128x128
block-diagonal / anti-diagonal stationary matrices, so every matmul
uses the full PE array with K=128.

Five-stage software pipeline (L1@c, L2@c+1, L3@c+3, L4@c+4) with one
PSUM eviction per engine per step: ACT does relu (z1->a1, f32r), Pool
does the s2 mask (is_gt, fp16), DVE does the fused (a1>0)*pre1 multiply
(fp16). The per-pair output eviction is split 320/192 between ACT and
DVE (DVE half staggered a step) to stay inside the 854ns PE block
budget. Inputs ship as fp16 (halves HBM traffic; masks lose ~1 bit vs
f32r), stationaries S1/S2 stay f32r, the value path (S3/S4/t1/out) is
fp16. The whole input is DMA'd up front in graduated chunks; weights
ride in two fused tensors (S12, S34) to cut HWDGE serialization; the
final four blocks are 256 wide so the drain chain is short.
"""

import sys

sys.path.insert(0, "/opt/trn_rl_repo")

from contextlib import ExitStack

import numpy as np

B, D, H = 262144, 32, 64
NCORES = 8
BC = B // NCORES          # samples per core
G = BC // 2               # samples per group (2 groups on 128 partitions)
CHUNK = 512               # batch columns per pipeline block (per group)

_CACHE = {}


def _round_f32r(a):
    """IEEE fp32 -> e8m11 (float32r): round mantissa to 11 bits (RNE)."""
    u = np.ascontiguousarray(a, np.float32).view(np.uint32)
    lsb = (u >> np.uint32(12)) & np.uint32(1)
    u2 = (u + np.uint32(0x7FF) + lsb) & np.uint32(0xFFFFF000)
    return u2.view(np.float32)


# input DMA chunk widths (columns); graduated so early blocks start early
XPLAN = [512, 512, 512, 512, 1024, 1024,
         2048, 2048, 2048, 2048, 2048, 1024, 1024]
# pipeline block widths; last four narrow to shorten the drain chain
WIDTHS = [512] * 30 + [256] * 4
N_TAIL_PAIRS = 3  # last pairs stored per-pair on the ACT ring


def _block_tables():
    starts = [0]
    for w in WIDTHS[:-1]:
        starts.append(starts[-1] + w)
    pair_w = [WIDTHS[2 * p] for p in range(len(WIDTHS) // 2)]
    pair_off = [0]
    for w in pair_w[:-1]:
        pair_off.append(pair_off[-1] + w)
    return starts, pair_w, pair_off


def _build(bc, chunk, x_dt="f32r", warm=26, e4a=320, bufs=None):
    import concourse.bass as bass
    import concourse.tile as tile
    from concourse import bacc, mybir

    f32 = mybir.dt.float32
    f32r = mybir.dt.float32r
    fp16 = mybir.dt.float16
    bf16 = mybir.dt.bfloat16
    Relu = mybir.ActivationFunctionType.Relu
    Copy = mybir.ActivationFunctionType.Copy
    is_gt = mybir.AluOpType.is_gt
    mult = mybir.AluOpType.mult
    bypass = mybir.AluOpType.bypass

    xdt = fp16 if x_dt == "fp16" else f32r

    g = bc // 2
    nb = len(WIDTHS)
    npairs = nb // 2
    assert sum(XPLAN) == g and sum(WIDTHS) == g
    starts, pair_w, pair_off = _block_tables()
    # block -> (chunk index, offset within chunk)
    blk_chunk = []
    ci, coff = 0, 0
    for w in WIDTHS:
        if coff >= XPLAN[ci]:
            ci += 1
            coff = 0
        blk_chunk.append((ci, coff))
        coff += w
    BUFS = {"a1": 6, "s2": 6, "t1": 5, "ot": 4}
    if bufs:
        BUFS.update(bufs)

    nc = bacc.Bacc("TRN2", target_bir_lowering=False, debug=False)

    # xT rows: p = grp*64 + f (group grp's feature f); cols: samples in group
    xT = nc.dram_tensor("xT", [128, g], xdt, kind="ExternalInput").ap()
    # S1 | S2 | b1cat | -b2cat fused so one DMA carries the L1/L2 constants
    S12 = nc.dram_tensor("S12", [128, 258], f32r, kind="ExternalInput").ap()
    S34 = nc.dram_tensor("S34", [128, 192], fp16, kind="ExternalInput").ap()
    # outT rows (blocks of 32): A-even / B-even / A-odd / B-odd block outputs;
    # cols: pair_off[p] + col
    outT = nc.dram_tensor("outT", [128, g // 2], fp16, kind="ExternalOutput").ap()

    with tile.TileContext(nc) as tc, ExitStack() as ctx:
        wp = ctx.enter_context(tc.tile_pool(name="w", bufs=1))
        xs_p = ctx.enter_context(tc.tile_pool(name="xs", bufs=1))
        a1_p = ctx.enter_context(tc.tile_pool(name="a1", bufs=BUFS["a1"]))
        s2_p = ctx.enter_context(tc.tile_pool(name="s2", bufs=BUFS["s2"]))
        t1_p = ctx.enter_context(tc.tile_pool(name="t1", bufs=BUFS["t1"]))
        ot_p = ctx.enter_context(tc.tile_pool(name="ot", bufs=BUFS["ot"]))
        pz1 = ctx.enter_context(tc.tile_pool(name="pz1", bufs=2, space="PSUM"))
        pz2 = ctx.enter_context(tc.tile_pool(name="pz2", bufs=2, space="PSUM"))
        pp1 = ctx.enter_context(tc.tile_pool(name="pp1", bufs=2, space="PSUM"))
        pout = ctx.enter_context(tc.tile_pool(name="po", bufs=2, space="PSUM"))

        s12_t = wp.tile([128, 258], f32r, tag="s12")
        s34_t = wp.tile([128, 192], fp16, tag="s34")

        xs_tiles = []
        for k, w_ in enumerate(XPLAN):
            xs_tiles.append(xs_p.tile([128, w_], xdt, tag=f"xs{k}",
                                      name=f"xs{k}"))
        xoff = [0]
        for w_ in XPLAN[:-1]:
            xoff.append(xoff[-1] + w_)

        # dum memset early so the dummy activation (ACT table preload) can
        # run right after the first scalar-queue DMA issue
        dum = wp.tile([128, 4], f32, tag="dum")
        wjunk = wp.tile([128, 128], bf16, tag="wjunk")
        nc.gpsimd.memset(dum[:], 0.0)
        nc.gpsimd.memset(wjunk[:], 0.0)

        # early DMA order tuned so block k's data lands just before L1(k):
        # SP carries S12 then odd chunks; scalar carries xs0, the ACT-table
        # preload, xs2, then S34 (needed only by L3(0) ~3 steps in)
        nc.sync.dma_start(out=s12_t[:], in_=S12)
        nc.scalar.dma_start(out=xs_tiles[0][:],
                            in_=xT[:, xoff[0]:xoff[0] + XPLAN[0]])
        # absorbs the one-time LoadActFuncSet (~1.3us) off the critical path
        nc.scalar.activation(out=dum[:], in_=dum[:], func=Relu, scale=1.0)
        nc.sync.dma_start(out=xs_tiles[1][:],
                          in_=xT[:, xoff[1]:xoff[1] + XPLAN[1]])
        nc.scalar.dma_start(out=xs_tiles[2][:],
                            in_=xT[:, xoff[2]:xoff[2] + XPLAN[2]])
        nc.sync.dma_start(out=xs_tiles[3][:],
                          in_=xT[:, xoff[3]:xoff[3] + XPLAN[3]])
        nc.scalar.dma_start(out=s34_t[:], in_=S34)
        for k in range(4, len(XPLAN)):
            nc.sync.dma_start(out=xs_tiles[k][:],
                              in_=xT[:, xoff[k]:xoff[k] + XPLAN[k]])
        s1w = s12_t[:, 0:128]
        s2w = s12_t[:, 128:256]
        s3w = s34_t[:, 0:128]
        s4w = s34_t[:, 128:192]
        bia = s12_t[:, 256:258].bitcast(f32)

        # PE warm-up: junk bf16 matmuls (results never read) advance the
        # clock-gate ramp so real matmuls hit 2.4 GHz once the first chunk
        # lands.
        warm_t = pz1.tile([128, chunk], f32, tag="pz1", name="warm")
        for _ in range(warm):
            nc.tensor.matmul(warm_t[:, 0:128], lhsT=wjunk[:], rhs=wjunk[:],
                             start=True, stop=True)

        a1s = {}
        s2ms = {}
        t1s = {}
        ots = {}
        e4b_q = []
        dma_q = []

        # Five-stage pipeline; per step c the PE stream is
        #   L1(c), L2(c-1), L3(c-3), L4(c-4)
        for c in range(nb + 4):
            # staggered DVE half of the previous pair's output eviction
            while e4b_q:
                op_, sl, w = e4b_q.pop()
                nc.vector.tensor_scalar(out=sl, in0=op_[:, e4a:w],
                                        scalar1=0.0, scalar2=None, op0=bypass)
            while dma_q:
                po_, ot_, wtot = dma_q.pop()
                nc.sync.dma_start(out=outT[:, po_:po_ + wtot],
                                  in_=ot_[:, 0:wtot])
            if c < nb:
                w = WIDTHS[c]
                ci, coff = blk_chunk[c]
                xs = xs_tiles[ci][:, coff:coff + w]
                z1p = pz1.tile([128, chunk], f32, tag="pz1", name="z1p")
                nc.tensor.matmul(z1p[:, 0:w], lhsT=s1w, rhs=xs,
                                 start=True, stop=True)
                a1 = a1_p.tile([128, chunk], f32r, tag="a1", name="a1")
                nc.scalar.activation(out=a1[:, 0:w], in_=z1p[:, 0:w],
                                     func=Relu, bias=bia[:, 0:1], scale=1.0)
                a1s[c] = a1

            if 0 <= c - 1 < nb:
                i = c - 1
                w = WIDTHS[i]
                z2p = pz2.tile([128, chunk], f32, tag="pz2", name="z2p")
                nc.tensor.matmul(z2p[:, 0:w], lhsT=s2w, rhs=a1s[i][:, 0:w],
                                 start=True, stop=True)
                s2m = s2_p.tile([128, chunk], fp16, tag="s2", name="s2m")
                nc.gpsimd.tensor_scalar(out=s2m[:, 0:w], in0=z2p[:, 0:w],
                                        scalar1=bia[:, 1:2], scalar2=None,
                                        op0=is_gt)
                s2ms[i] = s2m

            if 0 <= c - 3 < nb:
                i = c - 3
                w = WIDTHS[i]
                p1p = pp1.tile([128, chunk], f32, tag="pp1", name="p1p")
                nc.tensor.matmul(p1p[:, 0:w], lhsT=s3w,
                                 rhs=s2ms.pop(i)[:, 0:w],
                                 start=True, stop=True)
                t1 = t1_p.tile([128, chunk], fp16, tag="t1", name="t1")
                nc.vector.scalar_tensor_tensor(
                    out=t1[:, 0:w], in0=a1s.pop(i)[:, 0:w].bitcast(f32),
                    scalar=0.0, in1=p1p[:, 0:w], op0=is_gt, op1=mult)
                t1s[i] = t1

            if 0 <= c - 4 < nb:
                i = c - 4
                w = WIDTHS[i]
                par = i % 2
                pair = i // 2
                if par == 0:
                    ots[pair] = pout.tile([128, chunk], f32, tag="po",
                                          name="outp")
                op_ = ots[pair]
                nc.tensor.matmul(op_[64 * par:64 * (par + 1), 0:w],
                                 lhsT=s4w, rhs=t1s.pop(i)[:, 0:w],
                                 start=True, stop=True)
                if par == 1:
                    tail = pair >= npairs - N_TAIL_PAIRS
                    if tail:
                        # drain: full per-pair eviction on ACT/Pool (their
                        # steady streams have ended) and an immediate store
                        # on SP so the ACT queue never blocks behind HWDGE
                        ott = ot_p.tile([128, 2 * chunk], fp16, tag="ot",
                                        name="ott")
                        if pair == npairs - 2:
                            nc.gpsimd.tensor_scalar(out=ott[:, 0:w],
                                                    in0=op_[:, 0:w],
                                                    scalar1=0.0, scalar2=None,
                                                    op0=bypass)
                        else:
                            nc.scalar.activation(out=ott[:, 0:w],
                                                 in_=op_[:, 0:w], func=Copy)
                        nc.sync.dma_start(
                            out=outT[:, pair_off[pair]:pair_off[pair] + w],
                            in_=ott[:, 0:w])
                    else:
                        if pair % 2 == 0:
                            ots["sb"] = ot_p.tile([128, 2 * chunk], fp16,
                                                  tag="ot", name="ot")
                        ot = ots["sb"]
                        base = (pair % 2) * chunk
                        nc.scalar.activation(out=ot[:, base:base + e4a],
                                             in_=op_[:, 0:e4a], func=Copy)
                        e4b_q.append((op_, ot[:, base + e4a:base + w], w))
                        if pair % 2 == 1:
                            dma_q.append((pair_off[pair - 1], ot, 2 * chunk))
                    del ots[pair]

    nc.compile()
    return nc


def _get_nc(bc=BC, chunk=CHUNK, **kw):
    key = (bc, chunk, str(kw))
    if key not in _CACHE:
        _CACHE[key] = _build(bc, chunk, **kw)
    return _CACHE[key]


def _host_prep(W1, b1, W2, b2, W3, b3):
    w3 = np.asarray(W3)[:, 0].astype(np.float32)
    W1 = np.asarray(W1, np.float32)
    W2 = np.asarray(W2, np.float32)
    b1 = np.asarray(b1, np.float32)
    b2 = np.asarray(b2, np.float32)

    S12 = np.zeros((128, 258), np.float32)
    S12[:64, 0:64] = W1
    S12[64:, 64:128] = W1
    S12[:64, 192:256] = W2
    S12[64:, 128:192] = W2
    S12[:, 256] = np.concatenate([b1, b1])
    S12[:, 257] = -np.concatenate([b2, b2])
    S3s = (W2 * w3[None, :]).T  # [j, i] = w3[j] * W2[i, j]
    S34 = np.zeros((128, 192), np.float32)
    S34[64:, 0:64] = S3s    # A: s2 at p64:128 -> pre1 at p0:64
    S34[:64, 64:128] = S3s  # B: s2 at p0:64   -> pre1 at p64:128
    S4s = -(W1[:32, :].T)   # [64, 32]
    S34[:64, 128:160] = S4s  # A: t1 p0:64   -> out p0:32 (+64 for odd blocks)
    S34[64:, 160:192] = S4s  # B: t1 p64:128 -> out p32:64 (+64 for odd blocks)
    return {
        "S12": _round_f32r(S12),
        "S34": S34.astype(np.float16),
    }


def kernel(inputs, W1, b1, W2, b2, W3, b3):
    from concourse.bass_utils import run_bass_kernel_spmd

    x = np.ascontiguousarray(np.asarray(inputs, np.float32))
    consts = _host_prep(W1, b1, W2, b2, W3, b3)

    in_maps = []
    for k in range(NCORES):
        xc = x[k * BC:(k + 1) * BC]          # [BC, 64]
        # rows p = grp*64+f: group A samples [0,G) then group B [G,2G)
        xTk = _round_f32r(np.ascontiguousarray(
            np.concatenate([xc[:G].T, xc[G:].T], axis=0)))
        in_maps.append({"xT": xTk, **consts})

    nc = _get_nc()
    res = run_bass_kernel_spmd(nc, in_maps, core_ids=list(range(NCORES)),
                               trace=False)
    starts, pair_w, pair_off = _block_tables()
    outs = []
    for k in range(NCORES):
        oT = np.asarray(res.results[k]["outT"]).astype(np.float32)
        a = np.empty((G, 32), np.float32)
        b = np.empty((G, 32), np.float32)
        for p, w in enumerate(pair_w):
            blk = oT[:, pair_off[p]:pair_off[p] + w]
            se, so = starts[2 * p], starts[2 * p + 1]
            # rows: 4 groups of 32 = A-even / B-even / A-odd / B-odd
            a[se:se + w] = blk[0:32].T
            b[se:se + w] = blk[32:64].T
            a[so:so + w] = blk[64:96].T
            b[so:so + w] = blk[96:128].T
        outs.append(a)
        outs.append(b)
    out = np.concatenate(outs, axis=0).astype(np.float32)
    kernel._last_result = res
    return out


# revision 23
# speedup vs baseline: 1.0390x; 1.0390x over previous
"""Trainium2 Bass kernel for the LNN Euler-Lagrange residual.

Math: for a ReLU MLP Lagrangian L(q, qdot) the JAX second-derivative term
d/dt(dL/dqdot) is identically zero (piecewise-linear network), so the
reference output reduces to -dL/dq:

    z1 = x @ W1 + b1          s1 = z1 > 0      a1 = relu(z1)
    z2 = a1 @ W2 + b2         s2 = z2 > 0
    pre1 = s2 @ W2T_eff       (W2T_eff[j,i] = w3[j] * W2[i, j])
    out  = (pre1 * s1) @ (-W1[:32,:].T)

Layout: feature-major (features on partitions, batch streams as matmul
free dim). Host pre-transposes the input shard to [64, B_core]. Two
batch groups are packed on the 128 partitions via host-built 128x128
block-diagonal / anti-diagonal stationary matrices, so every matmul
uses the full PE array with K=128.

Five-stage software pipeline (L1@c, L2@c+1, L3@c+3, L4@c+4) with one
PSUM eviction per engine per step: ACT does relu (z1->a1, f32r), Pool
does the s2 mask (is_gt, fp16), DVE does the fused (a1>0)*pre1 multiply
(fp16). The per-pair output eviction is split 320/192 between ACT and
DVE (DVE half staggered a step) to stay inside the 854ns PE block
budget. Inputs ship as fp16 (halves HBM traffic; masks lose ~1 bit vs
f32r), stationaries S1/S2 stay f32r, the value path (S3/S4/t1/out) is
fp16. The whole input is DMA'd up front in graduated chunks; weights
ride in two fused tensors (S12, S34) to cut HWDGE serialization; the
final four blocks are 256 wide so the drain chain is short.
"""

import sys

sys.path.insert(0, "/opt/trn_rl_repo")

from contextlib import ExitStack

import numpy as np

B, D, H = 262144, 32, 64
NCORES = 8
BC = B // NCORES          # samples per core
G = BC // 2               # samples per group (2 groups on 128 partitions)
CHUNK = 512               # batch columns per pipeline block (per group)

_CACHE = {}


def _round_f32r(a):
    """IEEE fp32 -> e8m11 (float32r): round mantissa to 11 bits (RNE)."""
    u = np.ascontiguousarray(a, np.float32).view(np.uint32)
    lsb = (u >> np.uint32(12)) & np.uint32(1)
    u2 = (u + np.uint32(0x7FF) + lsb) & np.uint32(0xFFFFF000)
    return u2.view(np.float32)


# input DMA chunk widths (columns); graduated so early blocks start early,
# mostly-1024 so the f32r supply stream stays smooth vs consumption
XPLAN = [512, 512, 512, 512, 1024, 1024, 1024, 1024, 1024,
         1024, 1024, 1024, 1024, 1024, 2048, 2048]
# pipeline block widths; last four narrow to shorten the drain chain
WIDTHS = [512] * 30 + [256] * 4
N_TAIL_PAIRS = 3  # last pairs stored per-pair on the ACT ring


def _block_tables():
    starts = [0]
    for w in WIDTHS[:-1]:
        starts.append(starts[-1] + w)
    pair_w = [WIDTHS[2 * p] for p in range(len(WIDTHS) // 2)]
    pair_off = [0]
    for w in pair_w[:-1]:
        pair_off.append(pair_off[-1] + w)
    return starts, pair_w, pair_off


def _build(bc, chunk, x_dt="f32r", warm=28, e4a=320, bufs=None):
    import concourse.bass as bass
    import concourse.tile as tile
    from concourse import bacc, mybir

    f32 = mybir.dt.float32
    f32r = mybir.dt.float32r
    fp16 = mybir.dt.float16
    bf16 = mybir.dt.bfloat16
    Relu = mybir.ActivationFunctionType.Relu
    Copy = mybir.ActivationFunctionType.Copy
    is_gt = mybir.AluOpType.is_gt
    mult = mybir.AluOpType.mult
    bypass = mybir.AluOpType.bypass

    xdt = fp16 if x_dt == "fp16" else f32r

    g = bc // 2
    nb = len(WIDTHS)
    npairs = nb // 2
    assert sum(XPLAN) == g and sum(WIDTHS) == g
    starts, pair_w, pair_off = _block_tables()
    # block -> (chunk index, offset within chunk)
    blk_chunk = []
    ci, coff = 0, 0
    for w in WIDTHS:
        if coff >= XPLAN[ci]:
            ci += 1
            coff = 0
        blk_chunk.append((ci, coff))
        coff += w
    BUFS = {"a1": 8, "s2": 8, "t1": 6, "ot": 6}
    if bufs:
        BUFS.update(bufs)

    nc = bacc.Bacc("TRN2", target_bir_lowering=False, debug=False)

    # xT rows: p = grp*64 + f (group grp's feature f); cols: samples in group
    xT = nc.dram_tensor("xT", [128, g], xdt, kind="ExternalInput").ap()
    # S1 | S2 | b1cat | -b2cat fused so one DMA carries the L1/L2 constants
    S12 = nc.dram_tensor("S12", [128, 258], f32r, kind="ExternalInput").ap()
    S34 = nc.dram_tensor("S34", [128, 192], fp16, kind="ExternalInput").ap()
    # outT rows (blocks of 32): A-even / B-even / A-odd / B-odd block outputs;
    # cols: pair_off[p] + col
    outT = nc.dram_tensor("outT", [128, g // 2], fp16, kind="ExternalOutput").ap()

    with tile.TileContext(nc) as tc, ExitStack() as ctx:
        wp = ctx.enter_context(tc.tile_pool(name="w", bufs=1))
        xs_p = ctx.enter_context(tc.tile_pool(name="xs", bufs=1))
        a1_p = ctx.enter_context(tc.tile_pool(name="a1", bufs=BUFS["a1"]))
        s2_p = ctx.enter_context(tc.tile_pool(name="s2", bufs=BUFS["s2"]))
        t1_p = ctx.enter_context(tc.tile_pool(name="t1", bufs=BUFS["t1"]))
        ot_p = ctx.enter_context(tc.tile_pool(name="ot", bufs=BUFS["ot"]))
        pz1 = ctx.enter_context(tc.tile_pool(name="pz1", bufs=2, space="PSUM"))
        pz2 = ctx.enter_context(tc.tile_pool(name="pz2", bufs=2, space="PSUM"))
        pp1 = ctx.enter_context(tc.tile_pool(name="pp1", bufs=2, space="PSUM"))
        pout = ctx.enter_context(tc.tile_pool(name="po", bufs=2, space="PSUM"))

        s12_t = wp.tile([128, 258], f32r, tag="s12")
        s34_t = wp.tile([128, 192], fp16, tag="s34")

        xs_tiles = []
        for k, w_ in enumerate(XPLAN):
            xs_tiles.append(xs_p.tile([128, w_], xdt, tag=f"xs{k}",
                                      name=f"xs{k}"))
        xoff = [0]
        for w_ in XPLAN[:-1]:
            xoff.append(xoff[-1] + w_)

        # dum memset early so the dummy activation (ACT table preload) can
        # run right after the first scalar-queue DMA issue
        dum = wp.tile([128, 4], f32, tag="dum")
        wjunk = wp.tile([128, 128], bf16, tag="wjunk")
        nc.gpsimd.memset(dum[:], 0.0)
        nc.gpsimd.memset(wjunk[:], 0.0)

        # early DMA order tuned so block k's data lands just before L1(k):
        # SP carries S12 then odd chunks; scalar carries xs0, the ACT-table
        # preload, xs2, then S34 (needed only by L3(0) ~3 steps in)
        nc.sync.dma_start(out=s12_t[:], in_=S12)
        nc.scalar.dma_start(out=xs_tiles[0][:],
                            in_=xT[:, xoff[0]:xoff[0] + XPLAN[0]])
        # absorbs the one-time LoadActFuncSet (~1.3us) off the critical path
        nc.scalar.activation(out=dum[:], in_=dum[:], func=Relu, scale=1.0)
        nc.sync.dma_start(out=xs_tiles[1][:],
                          in_=xT[:, xoff[1]:xoff[1] + XPLAN[1]])
        nc.scalar.dma_start(out=xs_tiles[2][:],
                            in_=xT[:, xoff[2]:xoff[2] + XPLAN[2]])
        nc.sync.dma_start(out=xs_tiles[3][:],
                          in_=xT[:, xoff[3]:xoff[3] + XPLAN[3]])
        nc.scalar.dma_start(out=s34_t[:], in_=S34)
        for k in range(4, len(XPLAN)):
            nc.sync.dma_start(out=xs_tiles[k][:],
                              in_=xT[:, xoff[k]:xoff[k] + XPLAN[k]])
        s1w = s12_t[:, 0:128]
        s2w = s12_t[:, 128:256]
        s3w = s34_t[:, 0:128]
        s4w = s34_t[:, 128:192]
        bia = s12_t[:, 256:258].bitcast(f32)

        # PE warm-up: junk bf16 matmuls (results never read) advance the
        # clock-gate ramp so real matmuls hit 2.4 GHz once the first chunk
        # lands.
        warm_t = pz1.tile([128, chunk], f32, tag="pz1", name="warm")
        for _ in range(warm):
            nc.tensor.matmul(warm_t[:, 0:128], lhsT=wjunk[:], rhs=wjunk[:],
                             start=True, stop=True)

        a1s = {}
        s2ms = {}
        t1s = {}
        ots = {}
        e4b_q = []
        dma_q = []

        # Five-stage pipeline; per step c the PE stream is
        #   L1(c), L2(c-1), L3(c-3), L4(c-4)
        for c in range(nb + 4):
            # staggered DVE half of the previous pair's output eviction
            while e4b_q:
                op_, sl, w = e4b_q.pop()
                nc.vector.tensor_scalar(out=sl, in0=op_[:, e4a:w],
                                        scalar1=0.0, scalar2=None, op0=bypass)
            while dma_q:
                po_, ot_, wtot = dma_q.pop()
                nc.sync.dma_start(out=outT[:, po_:po_ + wtot],
                                  in_=ot_[:, 0:wtot])
            if c < nb:
                w = WIDTHS[c]
                ci, coff = blk_chunk[c]
                xs = xs_tiles[ci][:, coff:coff + w]
                z1p = pz1.tile([128, chunk], f32, tag="pz1", name="z1p")
                nc.tensor.matmul(z1p[:, 0:w], lhsT=s1w, rhs=xs,
                                 start=True, stop=True)
                a1 = a1_p.tile([128, chunk], f32r, tag="a1", name="a1")
                nc.scalar.activation(out=a1[:, 0:w], in_=z1p[:, 0:w],
                                     func=Relu, bias=bia[:, 0:1], scale=1.0)
                a1s[c] = a1

            if 0 <= c - 1 < nb:
                i = c - 1
                w = WIDTHS[i]
                z2p = pz2.tile([128, chunk], f32, tag="pz2", name="z2p")
                nc.tensor.matmul(z2p[:, 0:w], lhsT=s2w, rhs=a1s[i][:, 0:w],
                                 start=True, stop=True)
                s2m = s2_p.tile([128, chunk], fp16, tag="s2", name="s2m")
                nc.gpsimd.tensor_scalar(out=s2m[:, 0:w], in0=z2p[:, 0:w],
                                        scalar1=bia[:, 1:2], scalar2=None,
                                        op0=is_gt)
                s2ms[i] = s2m

            if 0 <= c - 3 < nb:
                i = c - 3
                w = WIDTHS[i]
                p1p = pp1.tile([128, chunk], f32, tag="pp1", name="p1p")
                nc.tensor.matmul(p1p[:, 0:w], lhsT=s3w,
                                 rhs=s2ms.pop(i)[:, 0:w],
                                 start=True, stop=True)
                t1 = t1_p.tile([128, chunk], fp16, tag="t1", name="t1")
                nc.vector.scalar_tensor_tensor(
                    out=t1[:, 0:w], in0=a1s.pop(i)[:, 0:w].bitcast(f32),
                    scalar=0.0, in1=p1p[:, 0:w], op0=is_gt, op1=mult)
                t1s[i] = t1

            if 0 <= c - 4 < nb:
                i = c - 4
                w = WIDTHS[i]
                par = i % 2
                pair = i // 2
                if par == 0:
                    ots[pair] = pout.tile([128, chunk], f32, tag="po",
                                          name="outp")
                op_ = ots[pair]
                nc.tensor.matmul(op_[64 * par:64 * (par + 1), 0:w],
                                 lhsT=s4w, rhs=t1s.pop(i)[:, 0:w],
                                 start=True, stop=True)
                if par == 1:
                    tail = pair >= npairs - N_TAIL_PAIRS
                    if tail:
                        # drain: full per-pair eviction on ACT/Pool (their
                        # steady streams have ended) and an immediate store
                        # on SP so the ACT queue never blocks behind HWDGE
                        ott = ot_p.tile([128, 2 * chunk], fp16, tag="ot",
                                        name="ott")
                        if pair == npairs - 2:
                            nc.gpsimd.tensor_scalar(out=ott[:, 0:w],
                                                    in0=op_[:, 0:w],
                                                    scalar1=0.0, scalar2=None,
                                                    op0=bypass)
                        else:
                            nc.scalar.activation(out=ott[:, 0:w],
                                                 in_=op_[:, 0:w], func=Copy)
                        nc.sync.dma_start(
                            out=outT[:, pair_off[pair]:pair_off[pair] + w],
                            in_=ott[:, 0:w])
                    else:
                        if pair % 2 == 0:
                            ots["sb"] = ot_p.tile([128, 2 * chunk], fp16,
                                                  tag="ot", name="ot")
                        ot = ots["sb"]
                        base = (pair % 2) * chunk
                        nc.scalar.activation(out=ot[:, base:base + e4a],
                                             in_=op_[:, 0:e4a], func=Copy)
                        e4b_q.append((op_, ot[:, base + e4a:base + w], w))
                        if pair % 2 == 1:
                            dma_q.append((pair_off[pair - 1], ot, 2 * chunk))
                    del ots[pair]

    nc.compile()
    return nc


def _get_nc(bc=BC, chunk=CHUNK, **kw):
    key = (bc, chunk, str(kw))
    if key not in _CACHE:
        _CACHE[key] = _build(bc, chunk, **kw)
    return _CACHE[key]


def _host_prep(W1, b1, W2, b2, W3, b3):
    w3 = np.asarray(W3)[:, 0].astype(np.float32)
    W1 = np.asarray(W1, np.float32)
    W2 = np.asarray(W2, np.float32)
    b1 = np.asarray(b1, np.float32)
    b2 = np.asarray(b2, np.float32)

    S12 = np.zeros((128, 258), np.float32)
    S12[:64, 0:64] = W1
    S12[64:, 64:128] = W1
    S12[:64, 192:256] = W2
    S12[64:, 128:192] = W2
    S12[:, 256] = np.concatenate([b1, b1])
    S12[:, 257] = -np.concatenate([b2, b2])
    S3s = (W2 * w3[None, :]).T  # [j, i] = w3[j] * W2[i, j]
    S34 = np.zeros((128, 192), np.float32)
    S34[64:, 0:64] = S3s    # A: s2 at p64:128 -> pre1 at p0:64
    S34[:64, 64:128] = S3s  # B: s2 at p0:64   -> pre1 at p64:128
    S4s = -(W1[:32, :].T)   # [64, 32]
    S34[:64, 128:160] = S4s  # A: t1 p0:64   -> out p0:32 (+64 for odd blocks)
    S34[64:, 160:192] = S4s  # B: t1 p64:128 -> out p32:64 (+64 for odd blocks)
    return {
        "S12": _round_f32r(S12),
        "S34": S34.astype(np.float16),
    }


def kernel(inputs, W1, b1, W2, b2, W3, b3):
    from concourse.bass_utils import run_bass_kernel_spmd

    x = np.ascontiguousarray(np.asarray(inputs, np.float32))
    consts = _host_prep(W1, b1, W2, b2, W3, b3)

    in_maps = []
    for k in range(NCORES):
        xc = x[k * BC:(k + 1) * BC]          # [BC, 64]
        # rows p = grp*64+f: group A samples [0,G) then group B [G,2G)
        xTk = _round_f32r(np.ascontiguousarray(
            np.concatenate([xc[:G].T, xc[G:].T], axis=0)))
        in_maps.append({"xT": xTk, **consts})

    nc = _get_nc()
    res = run_bass_kernel_spmd(nc, in_maps, core_ids=list(range(NCORES)),
                               trace=False)
    starts, pair_w, pair_off = _block_tables()
    outs = []
    for k in range(NCORES):
        oT = np.asarray(res.results[k]["outT"]).astype(np.float32)
        a = np.empty((G, 32), np.float32)
        b = np.empty((G, 32), np.float32)
        for p, w in enumerate(pair_w):
            blk = oT[:, pair_off[p]:pair_off[p] + w]
            se, so = starts[2 * p], starts[2 * p + 1]
            # rows: 4 groups of 32 = A-even / B-even / A-odd / B-odd
            a[se:se + w] = blk[0:32].T
            b[se:se + w] = blk[32:64].T
            a[so:so + w] = blk[64:96].T
            b[so:so + w] = blk[96:128].T
        outs.append(a)
        outs.append(b)
    out = np.concatenate(outs, axis=0).astype(np.float32)
    kernel._last_result = res
    return out
